# revision 26
# baseline (speedup 1.0000x reference)
"""Trainium2 Bass kernel for BinaryHead: logits = (l2norm(fea) @ W.T + b) * 16.

Sharding: data-parallel over the batch dim across 8 NeuronCores (2048 rows
each).  The host stages each core's shard TRANSPOSED ([emb, batch]) as bf16 so
the embedding/contraction dim lands on SBUF partitions.

v2: column-tiled PE.  The z matmuls have a 4-wide stationary (4 classes), so
a plain matmul uses 4/128 of the PE array and the kernel is PE-bound (each
moving column costs a cycle regardless of stationary width; measured 46us of
MATMUL on a 51us kernel).  Fix: assign panel p to array column-tile t=p%4 via
tile_position=(0,32t) -- four matmuls stream concurrently through disjoint
column groups of the array, cutting PE wall time ~4x so the kernel becomes
HBM-DMA bound (8 MiB bf16 shard / 358 GB/s ~= 23.4us).

Per core the device kernel streams e-panel groups [128 x 4panels x 2048b]:

  ztile[32t+c, b] += Wt_panel.T @ panel         (tile t, 4-col stationary)
  ztile[32t+4, b] += ones.T @ panel**2          (tile t, fp8 squares, col 4)

i.e. the per-tile z partials AND sumsq partials accumulate into disjoint
partition slices of the same 4 PSUM banks.  Epilogue per 512-col chunk:
gpsimd-copy the [128,512] bank to SBUF, then one [128,5] selection-matrix
matmul reduces the 4 tiles' z (cols 0-3) and sumsq (col 4) in a single pass;
rsqrt via exp(-0.5*ln(ss)+ln(S)) on ACT (partition 4), class-broadcast via a
k=1 matmul, then DVE mul + bias add and DMA out.  The first and last panel
groups are delivered in four column chunks: the first so the PE starts early,
the last so the epilogue pipelines into the tail of the stream.
"""

import os
from contextlib import ExitStack

import numpy as np

NUM_CLASS = 4
EMB = 2048
BATCH = 16384
N_CORES = 8
ROWS = BATCH // N_CORES  # 2048 rows per core
S = 16.0

N_ETILES = EMB // 128  # 16 e-panels per core
N_BCHUNK = ROWS // 512  # 4 psum-width chunks of the batch
CW = 512

# square-engine split: panel-equivalents on ACT vs DVE (tunable)
# middle (full) groups: per-panel map; chunked first/last groups: per-chunk map
ACT_SQ_PANELS = {5, 8, 11}  # panels in middle groups squared on ACT
ACT_SQ_CHUNKS_FIRST = {1, 2}  # chunks squared on ACT in the first group
ACT_SQ_CHUNKS_LAST = {1}  # chunks squared on ACT in the last group

DTYPE_CFG = "bf16"  # informational (test harness prints it)

_CACHE = {}


def _build_nc():
    import concourse.bacc as bacc
    import concourse.mybir as mybir
    import concourse.tile as tile
    from concourse.hw_specs import get_activation_tables

    f32 = mybir.dt.float32
    f32r = mybir.dt.float32r
    bf16 = mybir.dt.bfloat16
    fp8 = mybir.dt.float8e4

    nc = bacc.Bacc(
        "TRN2",
        target_bir_lowering=False,
        debug=False,
        enable_asserts=False,
        num_devices=N_CORES,
    )

    feaT = nc.dram_tensor("feaT", [EMB, ROWS], bf16, kind="ExternalInput").ap()
    wt = nc.dram_tensor(
        "wt", [128, N_ETILES * NUM_CLASS], bf16, kind="ExternalInput"
    ).ap()
    onesv = nc.dram_tensor("onesv", [128, 1], fp8, kind="ExternalInput").ap()
    # reduce stationaries: selz col c selects z of tile t (row 32t+c),
    # selss selects sumsq of tile t (row 32t)
    selz = nc.dram_tensor("selz", [128, NUM_CLASS], f32r, kind="ExternalInput").ap()
    # bf16: the f32r matmul path cannot target a non-zero column tile
    # (s3d3_mm_valid_dst_partition), and the Rs reduce writes col tile 1
    selss = nc.dram_tensor("selss", [128, 1], bf16, kind="ExternalInput").ap()
    sones = nc.dram_tensor("sones", [1, NUM_CLASS], f32r, kind="ExternalInput").ap()
    sbias = nc.dram_tensor("sbias", [NUM_CLASS, 1], f32, kind="ExternalInput").ap()
    outT = nc.dram_tensor("outT", [NUM_CLASS, ROWS], f32, kind="ExternalOutput").ap()

    with tile.TileContext(nc) as tc, ExitStack() as ctx:
        pconst = ctx.enter_context(tc.tile_pool(name="pconst", bufs=1))
        pdata = ctx.enter_context(tc.tile_pool(name="pdata", bufs=3))
        psq = ctx.enter_context(tc.tile_pool(name="psq", bufs=3))
        pep = ctx.enter_context(tc.tile_pool(name="pep", bufs=1))
        pzsb = ctx.enter_context(tc.tile_pool(name="pzsb", bufs=4))
        pz = ctx.enter_context(tc.tile_pool(name="pz", bufs=1, space="PSUM"))
        pr = ctx.enter_context(tc.tile_pool(name="pr", bufs=4, space="PSUM"))

        # wt/ones/sel ride the front of the sync ring (tiny transfers the
        # first matmuls need); tail-only consts go through SWDGE
        wt_s = pconst.tile([128, N_ETILES * NUM_CLASS], bf16)
        nc.sync.dma_start(out=wt_s, in_=wt)
        ones_s = pconst.tile([128, 1], fp8)
        nc.sync.dma_start(out=ones_s, in_=onesv)
        selz_s = pconst.tile([128, NUM_CLASS], f32r)
        nc.gpsimd.dma_start(out=selz_s, in_=selz)
        selss_s = pconst.tile([128, 1], bf16)
        nc.gpsimd.dma_start(out=selss_s, in_=selss)
        # sones placed on partition 32 (same partition as the reduced sumsq)
        sones_s = pconst.tile([128, NUM_CLASS], f32r)
        nc.gpsimd.dma_start(out=sones_s[32:33, :], in_=sones)
        sbias_s = pconst.tile([NUM_CLASS, 1], f32)
        nc.gpsimd.dma_start(out=sbias_s, in_=sbias)
        zero128_s = pconst.tile([128, 1], f32)
        nc.vector.memset(zero128_s, 0.0)
        # rsqrt via exp(-0.5*ln(ss) + ln(S)): folds the *S scale in for free
        lnS_s = pconst.tile([128, 1], f32)
        nc.vector.memset(lnS_s, float(np.log(S)))

        # z accumulator: one [128, 2048] 4-bank psum tensor; tile t's z
        # partial at partitions 32t..32t+3
        zt_ps = pz.tile([128, ROWS], f32, tag="zt")
        # sumsq accumulators: one single-bank [128, CW] tensor per chunk,
        # tile t's partial at partition 32t; the same 4 banks are then
        # recycled per chunk for {Rz@0-3 + Rs@32} and {rnb@0-3}
        ss_ps = [
            pr.tile([128, CW], f32, tag="rx", name=f"ss{j}") for j in range(N_BCHUNK)
        ]
        lnss_s = pep.tile([128, ROWS], f32)
        rnorm_s = pep.tile([128, ROWS], f32r)
        zr_s = pep.tile([NUM_CLASS, ROWS], f32)
        out_s = pep.tile([NUM_CLASS, ROWS], f32)

        def square(xt, x2, psl, bsl, eng):
            xin = xt[:, psl, bsl]
            if eng == "act":
                nc.scalar.activation(
                    out=x2[:, psl, bsl],
                    in_=xin,
                    func=mybir.ActivationFunctionType.Square,
                    bias=zero128_s,
                    scale=1.0,
                )
            else:
                nc.vector.tensor_mul(x2[:, psl, bsl], xin, xin)

        def z_mm(p, a, j, xt, start, stop):
            t = p % 4
            bsl = slice(j * CW, (j + 1) * CW)
            nc.tensor.matmul(
                zt_ps[32 * t : 32 * t + 4, bsl],
                wt_s[:, p * NUM_CLASS : (p + 1) * NUM_CLASS],
                xt[:, a, bsl],
                start=start,
                stop=stop,
                tile_position=(0, 32 * t),
            )

        def ss_mm(p, a, j, x2, start, stop):
            t = p % 4
            bsl = slice(j * CW, (j + 1) * CW)
            nc.tensor.matmul(
                ss_ps[j][32 * t : 32 * t + 1, :],
                ones_s,
                x2[:, a, bsl],
                start=start,
                stop=stop,
                tile_position=(0, 32 * t),
            )

        def copies_chunk(j):
            # evacuate psum to SBUF as soon as chunk j's accumulation stops
            # (GPSIMD has no PSUM access on TRN2, so DVE + ACT share the two)
            bsl = slice(j * CW, (j + 1) * CW)
            zsb = pzsb.tile([128, CW], f32r, tag="zsb", name=f"zsb{j}")
            nc.vector.tensor_copy(zsb, zt_ps[:, bsl])
            ssb = pzsb.tile([128, CW], bf16, tag="ssb", name=f"ssb{j}")
            nc.scalar.copy(ssb, ss_ps[j])
            return zsb, ssb

        def epilogue_chunk(j, zsb, ssb):
            # reduce 4 tiles' partials, then out.T = z*(S/sqrt(ss)) + S*b
            bsl = slice(j * CW, (j + 1) * CW)
            # bank_a: Rz at partitions 0-3 (col tile 0) + Rs at 32 (col tile 1)
            ra_ps = pr.tile([128, CW], f32, tag="rx", name=f"ra{j}")
            nc.tensor.matmul(
                ra_ps[0:NUM_CLASS, :],
                selz_s,
                zsb,
                start=True,
                stop=True,
                tile_position=(0, 0),
            )
            nc.tensor.matmul(
                ra_ps[32:33, :],
                selss_s,
                ssb,
                start=True,
                stop=True,
                tile_position=(0, 32),
            )
            nc.scalar.activation(
                out=lnss_s[32:33, bsl],
                in_=ra_ps[32:33, :],
                func=mybir.ActivationFunctionType.Ln,
                bias=zero128_s[32:33],
                scale=1.0,
            )
            nc.scalar.activation(
                out=rnorm_s[32:33, bsl],
                in_=lnss_s[32:33, bsl],
                func=mybir.ActivationFunctionType.Exp,
                bias=lnS_s[32:33],
                scale=-0.5,
            )
            # broadcast S/norm across the 4 class partitions via a k=1 f32r
            # matmul: array row tile 32 (rnorm lives on partition 32), col 0
            rb_ps = pr.tile([128, CW], f32, tag="rx", name=f"rb{j}")
            nc.tensor.matmul(
                rb_ps[0:NUM_CLASS, :],
                sones_s[32:33, :],
                rnorm_s[32:33, bsl],
                start=True,
                stop=True,
                tile_position=(32, 0),
            )
            # DVE can read only one PSUM operand per op: stage rnb in SBUF
            rnbs = pzsb.tile([NUM_CLASS, CW], f32, tag="rnbs", name=f"rnbs{j}")
            nc.scalar.copy(rnbs, rb_ps[0:NUM_CLASS, :])
            nc.vector.tensor_mul(zr_s[:, bsl], ra_ps[0:NUM_CLASS, :], rnbs)
            nc.vector.tensor_scalar_add(
                out_s[:, bsl], in0=zr_s[:, bsl], scalar1=sbias_s
            )
            nc.sync.dma_start(out=outT[:, bsl], in_=out_s[:, bsl])

        # pre-warm the PE while the first data transfer is in flight: dummy
        # matmuls into each tile's z region (the first real z matmul's
        # start=True resets those cells, so the garbage never survives)
        for w in range(24):
            t = w % 4
            nc.tensor.matmul(
                zt_ps[32 * t : 32 * t + 4, 0:64],
                wt_s[:, 0:NUM_CLASS],
                wt_s[:, 0:64],
                start=True,
                stop=True,
                tile_position=(0, 32 * t),
            )

        # 4 groups of 4 panels; first/last delivered in 4 column chunks
        groups = [tuple(range(4 * g, 4 * g + 4)) for g in range(4)]
        n_g = len(groups)
        xts, x2s = [None] * n_g, [None] * n_g

        def issue_dma(gi):
            g = groups[gi]
            xts[gi] = pdata.tile(
                [128, len(g), ROWS], bf16, tag="xt", name=f"xt{gi}"
            )
            x2s[gi] = psq.tile([128, len(g), ROWS], fp8, tag="x2", name=f"x2{gi}")
            src = feaT[g[0] * 128 : (g[-1] + 1) * 128, :].rearrange(
                "(a p) b -> p a b", p=128
            )
            # alternate the two HWDGE rings (SP and ACT) so transfers overlap
            dma_eng = nc.sync if gi % 2 == 0 else nc.scalar
            if gi == 0 or gi == n_g - 1:
                for j in range(N_BCHUNK):
                    bsl = slice(j * CW, (j + 1) * CW)
                    dma_eng.dma_start(out=xts[gi][:, :, bsl], in_=src[:, :, bsl])
            else:
                dma_eng.dma_start(out=xts[gi], in_=src)

        issue_dma(0)
        issue_dma(1)

        # ---- group 0 (column-chunked): squares + z matmuls per chunk ----
        for j in range(N_BCHUNK):
            bsl = slice(j * CW, (j + 1) * CW)
            square(xts[0], x2s[0], slice(None), bsl,
                   "act" if j in ACT_SQ_CHUNKS_FIRST else "dve")
            for p in groups[0]:
                z_mm(p, p % 4, j, xts[0], start=True, stop=False)

        # preload the ACT table set covering Square+Ln+Exp so no table
        # switch ever lands on the critical path
        nlx_id = list(get_activation_tables(nc.m.arch)).index(
            "natural_log_exp_and_others"
        )
        nc.scalar.add_instruction(
            mybir.InstLoadActFuncSet(name=f"I-{nc.next_id()}", act_func_set_id=nlx_id)
        )

        # ---- middle groups: z of group g, then ss of group g-1 (lag 1) ----
        for gi in (1, 2):
            issue_dma(gi + 1)
            g = groups[gi]
            # squares: DVE panels in pairs (saves per-op overhead), ACT singly
            a = 0
            while a < len(g):
                if g[a] in ACT_SQ_PANELS:
                    square(xts[gi], x2s[gi], slice(a, a + 1), slice(None), "act")
                    a += 1
                elif a + 1 < len(g) and g[a + 1] not in ACT_SQ_PANELS:
                    square(xts[gi], x2s[gi], slice(a, a + 2), slice(None), "dve")
                    a += 2
                else:
                    square(xts[gi], x2s[gi], slice(a, a + 1), slice(None), "dve")
                    a += 1
            for j in range(N_BCHUNK):
                for p in g:
                    z_mm(p, p % 4, j, xts[gi], start=False, stop=False)
                for p in groups[gi - 1]:
                    ss_mm(p, p % 4, j, x2s[gi - 1],
                          start=(gi == 1), stop=False)

        # ---- last group (column-chunked): ss and psum-evacuation copies lag
        # one chunk behind the stream, epilogues lag two (matches the psum
        # bank recycling: each chunk's Rz/Rs + rnb banks come from sumsq
        # banks freed by the copies one and two chunks earlier) ----
        gl = n_g - 1
        pend = [None] * N_BCHUNK
        for j in range(N_BCHUNK):
            bsl = slice(j * CW, (j + 1) * CW)
            square(xts[gl], x2s[gl], slice(None), bsl,
                   "act" if j in ACT_SQ_CHUNKS_LAST else "dve")
            for p in groups[gl]:
                z_mm(p, p % 4, j, xts[gl], start=False, stop=True)
            for p in groups[gl - 1]:
                ss_mm(p, p % 4, j, x2s[gl - 1], start=False, stop=False)
            if j > 0:
                for p in groups[gl]:
                    ss_mm(p, p % 4, j - 1, x2s[gl], start=False, stop=True)
                pend[j - 1] = copies_chunk(j - 1)
            if j > 1:
                epilogue_chunk(j - 2, *pend[j - 2])
        for p in groups[gl]:
            ss_mm(p, p % 4, N_BCHUNK - 1, x2s[gl], start=False, stop=True)
        pend[N_BCHUNK - 1] = copies_chunk(N_BCHUNK - 1)
        epilogue_chunk(N_BCHUNK - 2, *pend[N_BCHUNK - 2])
        epilogue_chunk(N_BCHUNK - 1, *pend[N_BCHUNK - 1])

    nc.compile()
    return nc


def _get_nc():
    if "nc" not in _CACHE:
        _CACHE["nc"] = _build_nc()
    return _CACHE["nc"]


def _stage_inputs(fea, W, b):
    import ml_dtypes

    fea = np.asarray(fea, dtype=np.float32)
    W = np.asarray(W, dtype=np.float32)
    b = np.asarray(b, dtype=np.float32)

    # wt[p, 4t+c] = W[c, 128t+p]
    wt = np.ascontiguousarray(
        W.reshape(NUM_CLASS, N_ETILES, 128).transpose(2, 1, 0).reshape(128, -1)
    ).astype(ml_dtypes.bfloat16)
    onesv = np.ones((128, 1), dtype=ml_dtypes.float8_e4m3)
    selz = np.zeros((128, NUM_CLASS), dtype=np.float32)
    selss = np.zeros((128, 1), dtype=ml_dtypes.bfloat16)
    for t in range(4):
        for c in range(NUM_CLASS):
            selz[32 * t + c, c] = 1.0
        selss[32 * t, 0] = 1.0
    # the *S scale is folded into the exp(-0.5*ln(ss)+ln(S)) rsqrt, so the
    # class-broadcast matmul uses plain ones
    sones = np.ones((1, NUM_CLASS), dtype=np.float32)
    sbias = (S * b).reshape(NUM_CLASS, 1).astype(np.float32)

    in_maps = []
    for i in range(N_CORES):
        shard = fea[i * ROWS : (i + 1) * ROWS, :]
        feaT = np.ascontiguousarray(shard.T).astype(ml_dtypes.bfloat16)
        in_maps.append(
            {
                "feaT": feaT,
                "wt": wt,
                "onesv": onesv,
                "selz": selz,
                "selss": selss,
                "sones": sones,
                "sbias": sbias,
            }
        )
    return in_maps


def run(fea, W, b, trace=False):
    from concourse.bass_utils import run_bass_kernel_spmd

    nc = _get_nc()
    in_maps = _stage_inputs(fea, W, b)
    res = run_bass_kernel_spmd(nc, in_maps, core_ids=list(range(N_CORES)), trace=trace)
    out = np.empty((BATCH, NUM_CLASS), dtype=np.float32)
    for i in range(N_CORES):
        out[i * ROWS : (i + 1) * ROWS, :] = res.results[i]["outT"].T
    return out, res


def kernel(fea, W, b):
    out, _ = run(fea, W, b, trace=False)
    return out


# revision 27
# speedup vs baseline: 1.2196x; 1.2196x over previous
"""Trainium2 Bass kernel for BinaryHead: logits = (l2norm(fea) @ W.T + b) * 16.

Sharding: data-parallel over the batch dim across 8 NeuronCores (2048 rows
each).  The host stages each core's shard TRANSPOSED ([emb, batch]) as bf16 so
the embedding/contraction dim lands on SBUF partitions, and batch-QUARTER
major ([quarter, panelgroup, 128, 4, 512] contiguous) so every DMA transfer
is a 512 KiB block with 4 KiB per-partition runs.

v3: column-tiled PE + chunk-major streaming.  The z matmuls have a 4-wide
stationary (4 classes), so a plain matmul uses 4/128 of the PE array and the
kernel is PE-bound.  Fix 1: panel p runs on array column-tile t=p%4 via
tile_position=(0,32t) -- four matmuls stream concurrently through disjoint
column groups, cutting PE wall time ~4x.  Fix 2: the batch streams in four
512-col quarters, 512 KiB per (quarter, panelgroup) transfer, so PE/DVE/ACT
work arrives every ~1.5us (no >3.4us PE-idle gaps -> the HAM clock-gate stays
at full rate) and each quarter's accumulation finishes while the next quarter
streams -- the normalization epilogue hides under the stream instead of
serializing at the kernel tail.

Per (quarter, panelgroup):  z_t += Wt.T @ x  (col tile t),  ss_t += 1.T @ x^2
with squares split across DVE/ACT/GPSIMD (bf16).  Per-quarter epilogue:
evacuate the two accumulator banks to SBUF (f32r/bf16), reduce the 4 tiles'
partials with tiny selection-matrix matmuls (Rz at col tile 0, Rs at col tile
1), rsqrt via exp(-0.5*ln(ss)+ln(S)) on ACT, class-broadcast via a k=1 matmul
(row tile 32), then DVE mul + bias add and DMA out on the scalar ring.
"""

from contextlib import ExitStack

import numpy as np

NUM_CLASS = 4
EMB = 2048
BATCH = 16384
N_CORES = 8
ROWS = BATCH // N_CORES  # 2048 rows per core
S = 16.0

N_ETILES = EMB // 128  # 16 e-panels per core
N_G = 4  # panel groups of 4
N_Q = 4  # batch quarters
CW = 512  # quarter width (one psum bank)

# square-engine map per (quarter, group): ACT and GPSIMD cells; rest on DVE.
# Quarter 0 avoids ACT so the activation-table preload runs first.
ACT_SQ = {(1, 1), (2, 1), (3, 1), (1, 2), (3, 2)}
GPS_SQ = {(0, 1), (0, 2), (2, 2)}

DTYPE_CFG = "bf16"  # informational (test harness prints it)

_CACHE = {}


def _build_nc():
    import concourse.bacc as bacc
    import concourse.mybir as mybir
    import concourse.tile as tile
    from concourse.hw_specs import get_activation_tables

    f32 = mybir.dt.float32
    f32r = mybir.dt.float32r
    bf16 = mybir.dt.bfloat16

    nc = bacc.Bacc(
        "TRN2",
        target_bir_lowering=False,
        debug=False,
        enable_asserts=False,
        num_devices=N_CORES,
    )

    feaT = nc.dram_tensor(
        "feaT", [N_Q, N_G, 128, 4, CW], bf16, kind="ExternalInput"
    ).ap()
    wt = nc.dram_tensor(
        "wt", [128, N_ETILES * NUM_CLASS], bf16, kind="ExternalInput"
    ).ap()
    onesv = nc.dram_tensor("onesv", [128, 1], bf16, kind="ExternalInput").ap()
    selz = nc.dram_tensor("selz", [128, NUM_CLASS], f32r, kind="ExternalInput").ap()
    # bf16: the f32r matmul path cannot target a non-zero column tile
    # (s3d3_mm_valid_dst_partition), and the Rs reduce writes col tile 1
    selss = nc.dram_tensor("selss", [128, 1], bf16, kind="ExternalInput").ap()
    sones = nc.dram_tensor("sones", [1, NUM_CLASS], f32r, kind="ExternalInput").ap()
    sbias = nc.dram_tensor("sbias", [NUM_CLASS, 1], f32, kind="ExternalInput").ap()
    outT = nc.dram_tensor("outT", [NUM_CLASS, ROWS], f32, kind="ExternalOutput").ap()

    with tile.TileContext(nc) as tc, ExitStack() as ctx:
        pconst = ctx.enter_context(tc.tile_pool(name="pconst", bufs=1))
        pdata = ctx.enter_context(tc.tile_pool(name="pdata", bufs=6))
        psq = ctx.enter_context(tc.tile_pool(name="psq", bufs=6))
        pep = ctx.enter_context(tc.tile_pool(name="pep", bufs=1))
        pev = ctx.enter_context(tc.tile_pool(name="pev", bufs=2))
        pz = ctx.enter_context(tc.tile_pool(name="pz", bufs=2, space="PSUM"))
        ps = ctx.enter_context(tc.tile_pool(name="ps", bufs=2, space="PSUM"))
        pr = ctx.enter_context(tc.tile_pool(name="pr", bufs=4, space="PSUM"))

        # all consts ride SWDGE so the sync HWDGE ring carries only the
        # 16 input-data transfers (first data block starts immediately)
        wt_s = pconst.tile([128, N_ETILES * NUM_CLASS], bf16)
        nc.gpsimd.dma_start(out=wt_s, in_=wt)
        ones_s = pconst.tile([128, 1], bf16)
        nc.gpsimd.dma_start(out=ones_s, in_=onesv)
        selz_s = pconst.tile([128, NUM_CLASS], f32r)
        nc.gpsimd.dma_start(out=selz_s, in_=selz)
        selss_s = pconst.tile([128, 1], bf16)
        nc.gpsimd.dma_start(out=selss_s, in_=selss)
        # sones placed on partition 32 (same partition as the reduced sumsq)
        sones_s = pconst.tile([128, NUM_CLASS], f32r)
        nc.gpsimd.dma_start(out=sones_s[32:33, :], in_=sones)
        sbias_s = pconst.tile([NUM_CLASS, 1], f32)
        nc.gpsimd.dma_start(out=sbias_s, in_=sbias)
        zero128_s = pconst.tile([128, 1], f32)
        nc.vector.memset(zero128_s, 0.0)
        # rsqrt via exp(-0.5*ln(ss) + ln(S)): folds the *S scale in for free
        lnS_s = pconst.tile([128, 1], f32)
        nc.vector.memset(lnS_s, float(np.log(S)))

        lnss_s = pep.tile([128, ROWS], f32)
        rnorm_s = pep.tile([128, ROWS], f32r)
        zr_s = pep.tile([NUM_CLASS, ROWS], f32)
        out_s = pep.tile([NUM_CLASS, ROWS], f32)

        # per-quarter accumulators, one psum bank each: tile t's z partial at
        # partitions 32t..32t+3, its sumsq partial at partition 32t
        zt_ps = [None] * N_Q
        ss_ps = [None] * N_Q
        xts = [[None] * N_G for _ in range(N_Q)]
        x2s = [[None] * N_G for _ in range(N_Q)]

        def issue_dma(j):
            for g in range(N_G):
                xts[j][g] = pdata.tile([128, 4, CW], bf16, tag="xt", name=f"xt{j}{g}")
                nc.sync.dma_start(out=xts[j][g], in_=feaT[j, g])

        def square(j, g):
            x2s[j][g] = psq.tile([128, 4, CW], bf16, tag="x2", name=f"x2{j}{g}")
            eng = (
                nc.scalar
                if (j, g) in ACT_SQ
                else nc.gpsimd
                if (j, g) in GPS_SQ
                else nc.vector
            )
            if eng is nc.scalar:
                nc.scalar.activation(
                    out=x2s[j][g],
                    in_=xts[j][g],
                    func=mybir.ActivationFunctionType.Square,
                    bias=zero128_s,
                    scale=1.0,
                )
            else:
                eng.tensor_mul(x2s[j][g], xts[j][g], xts[j][g])

        def z_mms(j, g, start, stop):
            for t in range(4):
                p = 4 * g + t
                nc.tensor.matmul(
                    zt_ps[j][32 * t : 32 * t + 4, :],
                    wt_s[:, p * NUM_CLASS : (p + 1) * NUM_CLASS],
                    xts[j][g][:, t, :],
                    start=start,
                    stop=stop,
                    tile_position=(0, 32 * t),
                )

        def ss_mms(j, g, start, stop):
            for t in range(4):
                nc.tensor.matmul(
                    ss_ps[j][32 * t : 32 * t + 1, :],
                    ones_s,
                    x2s[j][g][:, t, :],
                    start=start,
                    stop=stop,
                    tile_position=(0, 32 * t),
                )

        def epi_part1(j):
            # evacuate the two accumulator banks, reduce, ln
            bsl = slice(j * CW, (j + 1) * CW)
            zsb = pev.tile([128, CW], f32r, tag="zsb", name=f"zsb{j}")
            nc.vector.tensor_copy(zsb, zt_ps[j])
            ssb = pev.tile([128, CW], bf16, tag="ssb", name=f"ssb{j}")
            nc.scalar.copy(ssb, ss_ps[j])
            ra = pr.tile([128, CW], f32, tag="rx", name=f"ra{j}")
            nc.tensor.matmul(
                ra[0:NUM_CLASS, :],
                selz_s,
                zsb,
                start=True,
                stop=True,
                tile_position=(0, 0),
            )
            nc.tensor.matmul(
                ra[32:33, :],
                selss_s,
                ssb,
                start=True,
                stop=True,
                tile_position=(0, 32),
            )
            nc.scalar.activation(
                out=lnss_s[32:33, bsl],
                in_=ra[32:33, :],
                func=mybir.ActivationFunctionType.Ln,
                bias=zero128_s[32:33],
                scale=1.0,
            )
            nc.scalar.activation(
                out=rnorm_s[32:33, bsl],
                in_=lnss_s[32:33, bsl],
                func=mybir.ActivationFunctionType.Exp,
                bias=lnS_s[32:33],
                scale=-0.5,
            )
            return ra

        def epi_part2(j, ra):
            # rnorm broadcast (k=1 matmul on row tile 32), scale, bias, out
            bsl = slice(j * CW, (j + 1) * CW)
            rb = pr.tile([128, CW], f32, tag="rx", name=f"rb{j}")
            nc.tensor.matmul(
                rb[0:NUM_CLASS, :],
                sones_s[32:33, :],
                rnorm_s[32:33, bsl],
                start=True,
                stop=True,
                tile_position=(32, 0),
            )
            # DVE can read only one PSUM operand per op: stage rnb in SBUF
            rnbs = pev.tile([NUM_CLASS, CW], f32, tag="rnbs", name=f"rnbs{j}")
            nc.vector.tensor_copy(rnbs, rb[0:NUM_CLASS, :])
            nc.vector.tensor_mul(zr_s[:, bsl], ra[0:NUM_CLASS, :], rnbs)
            nc.vector.tensor_scalar_add(
                out_s[:, bsl], in0=zr_s[:, bsl], scalar1=sbias_s
            )
            nc.scalar.dma_start(out=outT[:, bsl], in_=out_s[:, bsl])

        issue_dma(0)
        # activation-table preload (Square+Ln+Exp in one set) while the ACT
        # queue is otherwise empty; quarter 0's squares avoid ACT
        nlx_id = list(get_activation_tables(nc.m.arch)).index(
            "natural_log_exp_and_others"
        )
        nc.scalar.add_instruction(
            mybir.InstLoadActFuncSet(name=f"I-{nc.next_id()}", act_func_set_id=nlx_id)
        )

        ras = [None] * N_Q
        for j in range(N_Q):
            zt_ps[j] = pz.tile([128, CW], f32, tag="zt", name=f"zt{j}")
            ss_ps[j] = ps.tile([128, CW], f32, tag="ss", name=f"ss{j}")
            if j == 0:
                # pre-warm the PE while the first transfer is in flight: the
                # first real z matmul's start=True resets the garbage
                for w in range(24):
                    t = w % 4
                    nc.tensor.matmul(
                        zt_ps[0][32 * t : 32 * t + 4, 0:64],
                        wt_s[:, 0:NUM_CLASS],
                        wt_s[:, 0:64],
                        start=True,
                        stop=True,
                        tile_position=(0, 32 * t),
                    )
            if j + 1 < N_Q:
                issue_dma(j + 1)
            for g in range(N_G):
                square(j, g)
                z_mms(j, g, start=(g == 0), stop=(g == N_G - 1))
                if g > 0:
                    ss_mms(j, g - 1, start=(g == 1), stop=False)
                if j > 0 and g == 1:
                    ras[j - 1] = epi_part1(j - 1)
                if j > 0 and g == 2:
                    epi_part2(j - 1, ras[j - 1])
            ss_mms(j, N_G - 1, start=False, stop=True)
        ras[N_Q - 1] = epi_part1(N_Q - 1)
        epi_part2(N_Q - 1, ras[N_Q - 1])

    nc.compile()
    return nc


def _get_nc():
    if "nc" not in _CACHE:
        _CACHE["nc"] = _build_nc()
    return _CACHE["nc"]


def _stage_inputs(fea, W, b):
    import ml_dtypes

    fea = np.asarray(fea, dtype=np.float32)
    W = np.asarray(W, dtype=np.float32)
    b = np.asarray(b, dtype=np.float32)

    # wt[p, 4t+c] = W[c, 128t+p]
    wt = np.ascontiguousarray(
        W.reshape(NUM_CLASS, N_ETILES, 128).transpose(2, 1, 0).reshape(128, -1)
    ).astype(ml_dtypes.bfloat16)
    onesv = np.ones((128, 1), dtype=ml_dtypes.bfloat16)
    selz = np.zeros((128, NUM_CLASS), dtype=np.float32)
    selss = np.zeros((128, 1), dtype=ml_dtypes.bfloat16)
    for t in range(4):
        for c in range(NUM_CLASS):
            selz[32 * t + c, c] = 1.0
        selss[32 * t, 0] = 1.0
    # the *S scale is folded into the exp(-0.5*ln(ss)+ln(S)) rsqrt, so the
    # class-broadcast matmul uses plain ones
    sones = np.ones((1, NUM_CLASS), dtype=np.float32)
    sbias = (S * b).reshape(NUM_CLASS, 1).astype(np.float32)

    in_maps = []
    for i in range(N_CORES):
        shard = fea[i * ROWS : (i + 1) * ROWS, :]
        feaT = np.ascontiguousarray(shard.T)  # [EMB, ROWS]
        # [quarter j, group g, partition p, panel a, col b]
        fea5 = np.ascontiguousarray(
            feaT.reshape(N_G, 4, 128, N_Q, CW).transpose(3, 0, 2, 1, 4)
        ).astype(ml_dtypes.bfloat16)
        in_maps.append(
            {
                "feaT": fea5,
                "wt": wt,
                "onesv": onesv,
                "selz": selz,
                "selss": selss,
                "sones": sones,
                "sbias": sbias,
            }
        )
    return in_maps


def run(fea, W, b, trace=False):
    from concourse.bass_utils import run_bass_kernel_spmd

    nc = _get_nc()
    in_maps = _stage_inputs(fea, W, b)
    res = run_bass_kernel_spmd(nc, in_maps, core_ids=list(range(N_CORES)), trace=trace)
    out = np.empty((BATCH, NUM_CLASS), dtype=np.float32)
    for i in range(N_CORES):
        out[i * ROWS : (i + 1) * ROWS, :] = res.results[i]["outT"].T
    return out, res


def kernel(fea, W, b):
    out, _ = run(fea, W, b, trace=False)
    return out
